# revision 52
# baseline (speedup 1.0000x reference)
"""RGCN (basis-decomposed relational GCN) forward on 8 Trainium2 NeuronCores.

Strategy: shard by destination node (2500 nodes/core). Host buckets+sorts each
core's incoming edges by (dst-block, relation, dst-within-block) and gathers
the source rows into slot order (bf16), so the device streams them densely --
no on-device gather (SWDGE dma_gather costs ~8 ns/row serial on GpSimd, a
~640 us/core floor). Blocks are processed in per-core descending-edge-count
order so all cores share a tight chunk schedule (max of order statistics
instead of elementwise max). On device, per 128-segment block, streamed rows
are scatter-added into PSUM via one-hot matmuls (one-hots built in one
is_equal per group from a broadcast segment-label read); a second fused
matmul stage, software-pipelined one group behind, contracts the
per-(node,relation) sums with the relation weights plus the root term.
No collectives: each core owns its output rows outright.
"""

import os
import sys

import numpy as np
import ml_dtypes

for _p in ("/opt/trn_rl_repo", "/root/.axon_site/_ro/trn_rl_repo"):
    if os.path.isdir(_p) and _p not in sys.path:
        sys.path.append(_p)

import concourse.bacc as bacc
import concourse.tile as tile
from concourse import mybir
from concourse.bass_utils import run_bass_kernel_spmd

BF16 = ml_dtypes.bfloat16
N, E, IN, OUT, R = 20000, 640000, 256, 800, 8
NCORES = 8
NPC = N // NCORES            # 2500 nodes per core
NPAD = 2560                  # padded to 20 groups of 128 nodes
BLOCKS = NPAD // 16          # 160 blocks of 16 nodes (= 128 segments each)
GROUPS = NPAD // 128         # 20
CHUNK = 128

_PROGRAM_CACHE = {}
LAST_RESULT = None           # test harness reads profiling info from here


def _build(chunks):
    """Compile the SPMD program for per-slot chunk counts (same on all cores)."""
    dt = mybir.dt
    nc = bacc.Bacc("TRN2", target_bir_lowering=False, debug=False,
                   enable_asserts=True, num_devices=NCORES)
    TOT = sum(chunks) * CHUNK
    gchs = [sum(chunks[g * 8:(g + 1) * 8]) for g in range(GROUPS)]
    GMAX = max(gchs)
    xg_d = nc.dram_tensor("xg", [128, (TOT // 128) * IN], dt.bfloat16,
                          kind="ExternalInput").ap()
    segl_d = nc.dram_tensor("segl", [128, TOT // 128], dt.bfloat16, kind="ExternalInput").ap()
    xT_d = nc.dram_tensor("xT", [128, 2 * NPAD], dt.bfloat16, kind="ExternalInput").ap()
    w_d = nc.dram_tensor("w", [128, R * 2 * OUT], dt.bfloat16, kind="ExternalInput").ap()
    root_d = nc.dram_tensor("root", [128, 2 * OUT], dt.bfloat16, kind="ExternalInput").ap()
    out_d = nc.dram_tensor("out", [NPAD, OUT], dt.float32, kind="ExternalOutput").ap()

    with tile.TileContext(nc) as tc:
        with tc.tile_pool(name="const", bufs=1) as cp, \
             tc.tile_pool(name="xgp", bufs=3) as xgp, \
             tc.tile_pool(name="ohp", bufs=3) as ohp, \
             tc.tile_pool(name="stp", bufs=3) as stp, \
             tc.tile_pool(name="outp", bufs=3) as outp, \
             tc.tile_pool(name="psp", bufs=2, space="PSUM") as psp, \
             tc.tile_pool(name="pso", bufs=2, space="PSUM") as pso:
            # segl + iota first: the group-0 one-hot depends only on these
            segl_sb = cp.tile([128, TOT // 128], dt.bfloat16)
            nc.sync.dma_start(segl_sb[:], segl_d[:, :])
            # iota_b[p, c*128+f] = f (contiguous, one block's worth), for is_equal
            BMAX = max(chunks)
            iota_i = cp.tile([128, BMAX * 128], dt.int32)
            nc.gpsimd.iota(iota_i[:], pattern=[[0, BMAX], [1, 128]], base=0,
                           channel_multiplier=0)
            iota_b = cp.tile([128, BMAX * 128], dt.bfloat16)
            nc.vector.tensor_copy(iota_b[:], iota_i[:])
            # weight tiles load interleaved with the first groups' xg (below):
            # stage 2 of group 0 is the first consumer, and loading them all
            # up front would delay stage 1
            xT_sb = cp.tile([128, 2 * NPAD], dt.bfloat16)
            w_sbs = []
            for r8 in range(R):
                w_r = cp.tile([128, 2 * OUT], dt.bfloat16, name=f"w_r{r8}")
                w_sbs.append(w_r)
            root_sb = cp.tile([128, 2 * OUT], dt.bfloat16)

            def stage2(g, st):
                out_ps = pso.tile([128, OUT], dt.float32, tag="ops")
                out_sb = outp.tile([128, OUT], dt.float32, tag="osb")
                for (fo, fl) in ((0, 512), (512, 288)):
                    for h in range(2):
                        nc.tensor.matmul(
                            out=out_ps[:, fo:fo + fl],
                            lhsT=xT_sb[:, h * NPAD + g * 128: h * NPAD + (g + 1) * 128],
                            rhs=root_sb[:, h * OUT + fo: h * OUT + fo + fl],
                            start=(h == 0), stop=False)
                    for r8 in range(R):
                        for h in range(2):
                            nc.tensor.matmul(
                                out=out_ps[:, fo:fo + fl],
                                lhsT=st[:, h * 1024 + r8 * 128: h * 1024 + (r8 + 1) * 128],
                                rhs=w_sbs[r8][:, h * OUT + fo: h * OUT + fo + fl],
                                start=False, stop=(r8 == R - 1 and h == 1))
                nc.scalar.copy(out_sb[:], out_ps[:])
                nc.sync.dma_start(out_d[g * 128:(g + 1) * 128, :], out_sb[:])

            gbase = 0  # running chunk index
            prev = None  # (g, st) awaiting stage2, one group behind
            for g in range(GROUPS):
                bchunks = chunks[g * 8:(g + 1) * 8]
                gch = gchs[g]
                # gathered source rows, slot-ordered: partition = slot % 128.
                # One tile per DMA split so stage-1 matmuls only wait for the
                # split they read, not the whole group load.
                nsplit = 8
                csz = -(-gch // nsplit)
                xgs = []
                for si in range(nsplit):
                    c0, c1 = si * csz, min((si + 1) * csz, gch)
                    if c0 >= c1:
                        break
                    xs = xgp.tile([128, (c1 - c0) * IN], dt.bfloat16, tag=f"xg{si}")
                    nc.sync.dma_start(
                        xs[:], xg_d[:, (gbase + c0) * IN:(gbase + c1) * IN])
                    xgs.append(xs)
                if g == 0:
                    # weights ride the scalar engine's HWDGE ring so the sync
                    # ring streams xg groups back-to-back during the fill
                    nc.scalar.dma_start(xT_sb[:], xT_d[:, :])
                    nc.scalar.dma_start(root_sb[:], root_d[:, :])
                    for r8 in range(R):
                        nc.scalar.dma_start(
                            w_sbs[r8][:],
                            w_d[:, r8 * 2 * OUT:(r8 + 1) * 2 * OUT])

                def xg_slice(ci, h):
                    si, cl = ci // csz, ci % csz
                    return xgs[si][:, cl * IN + h * 128: cl * IN + (h + 1) * 128]
                # st columns: h*1024 + r*128 + b8*16 + n  (h = input half)
                st = stp.tile([128, 2048], dt.bfloat16, tag="st")
                st5 = st[:].rearrange("p (h r bb n) -> p h r bb n", h=2, r=8, bb=8, n=16)
                coff = 0
                for b8 in range(8):
                    cb = bchunks[b8]
                    # per-block one-hot: oh[p, c, f] = (segl[p, coff+c] == f)
                    oh = ohp.tile([128, cb * 128], dt.bfloat16, tag=f"oh{b8}")
                    nc.vector.tensor_tensor(
                        out=oh[:].rearrange("p (c f) -> p c f", f=128),
                        in0=segl_sb[:, gbase + coff:gbase + coff + cb]
                            .to_broadcast([128, cb, 128]),
                        in1=iota_b[:, :cb * 128].rearrange("p (c f) -> p c f", f=128),
                        op=mybir.AluOpType.is_equal)
                    pss = []
                    for h in range(2):
                        ps_h = psp.tile([128, 128], dt.float32, tag=f"ps{h}")
                        pss.append(ps_h)
                    for ci in range(coff, coff + cb):
                        for h in range(2):
                            nc.tensor.matmul(
                                out=pss[h][:],
                                lhsT=xg_slice(ci, h),
                                rhs=oh[:, (ci - coff) * 128:(ci - coff + 1) * 128],
                                start=(ci == coff), stop=(ci == coff + cb - 1))
                    for h in range(2):
                        nc.scalar.copy(
                            st5[:, h, :, b8, :],
                            pss[h][:].rearrange("p (r n) -> p r n", r=8, n=16))
                    coff += cb
                # groups 0-1 run stage 2 immediately (pipeline still filling);
                # later groups lag one group so PE never waits on the copies
                if g < 2:
                    stage2(g, st)
                else:
                    if prev is not None:
                        stage2(*prev)
                    prev = (g, st)
                gbase += gch
            if prev is not None:
                stage2(*prev)
    nc.compile()
    return nc


def _prep_core(src, dst, et, core):
    """Per-core block permutation + sorted schedule ingredients."""
    dl = dst - core * NPC
    blk = (dl // 16).astype(np.int64)
    counts = np.bincount(blk, minlength=BLOCKS)
    # descending count: big stage-1 groups first keep PE fed during the
    # DMA pipeline fill; ranks align across cores either way
    perm = np.argsort(-counts, kind="stable")
    rank = np.empty(BLOCKS, np.int64)
    rank[perm] = np.arange(BLOCKS)
    key = rank[blk] * 128 + et.astype(np.int64) * 16 + (dl % 16)
    order = np.argsort(key, kind="stable")
    return counts[perm], perm, key, order


def kernel(x, edge_index, edge_type, bases, att, root, bias):
    global LAST_RESULT
    x = np.asarray(x, dtype=np.float32)
    edge_index = np.asarray(edge_index, dtype=np.int32)
    edge_type = np.asarray(edge_type, dtype=np.int32)
    bases = np.asarray(bases, dtype=np.float32)
    att = np.asarray(att, dtype=np.float32)
    root = np.asarray(root, dtype=np.float32)
    bias = np.asarray(bias, dtype=np.float32)

    src_all, dst_all = edge_index[0], edge_index[1]
    core_of = dst_all // NPC

    per_core, preps = [], []
    for c in range(NCORES):
        m = core_of == c
        pc = (src_all[m], dst_all[m], edge_type[m])
        per_core.append(pc)
        preps.append(_prep_core(pc[0], pc[1], pc[2], c))
    counts_sched = np.stack([p[0] for p in preps])          # [8, BLOCKS] desc
    chunks = np.maximum(1, -(-counts_sched // CHUNK)).max(0)
    chunks = tuple(int(v) for v in chunks)
    TOT = sum(chunks) * CHUNK
    block_slot_start = np.concatenate([[0], np.cumsum(np.asarray(chunks) * CHUNK)])

    key = (tuple(chunks),)
    if key not in _PROGRAM_CACHE:
        _PROGRAM_CACHE[key] = _build(chunks)
    nc = _PROGRAM_CACHE[key]

    # shared weights
    W = np.einsum("rb,bio->rio", att, bases).astype(np.float32)      # [R, IN, OUT]
    w_dev = np.ascontiguousarray(
        W.reshape(R, 2, 128, OUT).transpose(2, 0, 1, 3).reshape(128, R * 2 * OUT)
    ).astype(BF16)
    root_dev = np.ascontiguousarray(
        root.reshape(2, 128, OUT).transpose(1, 0, 2).reshape(128, 2 * OUT)
    ).astype(BF16)
    x_bf = np.ascontiguousarray(x).astype(BF16)

    in_maps, perms = [], []
    for c in range(NCORES):
        src, dst, et = per_core[c]
        counts_s, perm, key_s, order = preps[c]
        perms.append(perm)
        src_s = src[order]
        key_ss = key_s[order]
        slot16 = key_ss // 128                     # schedule position of block
        segl_s = (key_ss % 128).astype(np.float32)
        edge_block_start = np.concatenate([[0], np.cumsum(counts_s)])
        within = np.arange(len(src_s)) - edge_block_start[slot16]
        pos = block_slot_start[slot16] + within

        src_slots = np.zeros(TOT, np.int32)
        segl_slots = np.full(TOT, -1.0, np.float32)
        src_slots[pos] = src_s
        segl_slots[pos] = segl_s

        # partition-major layout: xg_dev[p, c*IN:(c+1)*IN] = x[src of slot c*128+p]
        xg_dev = x_bf[src_slots.reshape(-1, 128).T].reshape(128, -1)
        segl_dev = np.ascontiguousarray(
            segl_slots.reshape(-1, 128).T).astype(BF16)

        # xT rows follow the block schedule order (perm)
        pexp = (perm[:, None] * 16 + np.arange(16)[None, :]).ravel()  # [NPAD]
        xc = np.zeros((NPAD, IN), np.float32)
        xc[:NPC] = x[c * NPC:(c + 1) * NPC]
        xcp = xc[pexp]
        xT_dev = np.ascontiguousarray(
            xcp.reshape(NPAD, 2, 128).transpose(2, 1, 0).reshape(128, 2 * NPAD)
        ).astype(BF16)

        in_maps.append({
            "xg": xg_dev, "segl": segl_dev,
            "xT": xT_dev, "w": w_dev, "root": root_dev,
        })

    res = run_bass_kernel_spmd(nc, in_maps, core_ids=list(range(NCORES)))
    LAST_RESULT = res

    out = np.empty((N, OUT), np.float32)
    for c in range(NCORES):
        pexp = (perms[c][:, None] * 16 + np.arange(16)[None, :]).ravel()
        rows = res.results[c]["out"]               # [NPAD, OUT], schedule order
        mask = pexp < NPC
        out[c * NPC + pexp[mask]] = rows[mask]
    out += bias[None, :]
    return out


# revision 53
# speedup vs baseline: 1.0078x; 1.0078x over previous
"""RGCN (basis-decomposed relational GCN) forward on 8 Trainium2 NeuronCores.

Strategy: shard by destination node (2500 nodes/core). Host buckets+sorts each
core's incoming edges by (dst-block, relation, dst-within-block) and gathers
the source rows into slot order (bf16), so the device streams them densely --
no on-device gather (SWDGE dma_gather costs ~8 ns/row serial on GpSimd, a
~640 us/core floor). Blocks are processed in per-core descending-edge-count
order so all cores share a tight chunk schedule (max of order statistics
instead of elementwise max). On device, per 128-segment block, streamed rows
are scatter-added into PSUM via one-hot matmuls (one-hots built in one
is_equal per group from a broadcast segment-label read); a second fused
matmul stage, software-pipelined one group behind, contracts the
per-(node,relation) sums with the relation weights plus the root term.
No collectives: each core owns its output rows outright.
"""

import os
import sys

import numpy as np
import ml_dtypes

for _p in ("/opt/trn_rl_repo", "/root/.axon_site/_ro/trn_rl_repo"):
    if os.path.isdir(_p) and _p not in sys.path:
        sys.path.append(_p)

import concourse.bacc as bacc
import concourse.tile as tile
from concourse import mybir
from concourse.bass_utils import run_bass_kernel_spmd

BF16 = ml_dtypes.bfloat16
N, E, IN, OUT, R = 20000, 640000, 256, 800, 8
NCORES = 8
NPC = N // NCORES            # 2500 nodes per core
NPAD = 2560                  # padded to 20 groups of 128 nodes
BLOCKS = NPAD // 16          # 160 blocks of 16 nodes (= 128 segments each)
GROUPS = NPAD // 128         # 20
CHUNK = 128

_PROGRAM_CACHE = {}
LAST_RESULT = None           # test harness reads profiling info from here


def _build(chunks):
    """Compile the SPMD program for per-slot chunk counts (same on all cores)."""
    dt = mybir.dt
    nc = bacc.Bacc("TRN2", target_bir_lowering=False, debug=False,
                   enable_asserts=True, num_devices=NCORES)
    TOT = sum(chunks) * CHUNK
    gchs = [sum(chunks[g * 8:(g + 1) * 8]) for g in range(GROUPS)]
    GMAX = max(gchs)
    xg_d = nc.dram_tensor("xg", [128, (TOT // 128) * IN], dt.bfloat16,
                          kind="ExternalInput").ap()
    segl_d = nc.dram_tensor("segl", [128, TOT // 128], dt.bfloat16, kind="ExternalInput").ap()
    xT_d = nc.dram_tensor("xT", [128, 2 * NPAD], dt.bfloat16, kind="ExternalInput").ap()
    w_d = nc.dram_tensor("w", [128, R * 2 * OUT], dt.bfloat16, kind="ExternalInput").ap()
    root_d = nc.dram_tensor("root", [128, 2 * OUT], dt.bfloat16, kind="ExternalInput").ap()
    out_d = nc.dram_tensor("out", [NPAD, OUT], dt.float32, kind="ExternalOutput").ap()

    with tile.TileContext(nc) as tc:
        with tc.tile_pool(name="const", bufs=1) as cp, \
             tc.tile_pool(name="xgp", bufs=3) as xgp, \
             tc.tile_pool(name="ohp", bufs=3) as ohp, \
             tc.tile_pool(name="stp", bufs=3) as stp, \
             tc.tile_pool(name="outp", bufs=3) as outp, \
             tc.tile_pool(name="psp", bufs=2, space="PSUM") as psp, \
             tc.tile_pool(name="pso", bufs=2, space="PSUM") as pso:
            # segl + iota first: the group-0 one-hot depends only on these
            segl_sb = cp.tile([128, TOT // 128], dt.bfloat16)
            nc.sync.dma_start(segl_sb[:], segl_d[:, :])
            # iota_b[p, c*128+f] = f (contiguous, one block's worth), for is_equal
            BMAX = max(chunks)
            iota_i = cp.tile([128, BMAX * 128], dt.int32)
            nc.gpsimd.iota(iota_i[:], pattern=[[0, BMAX], [1, 128]], base=0,
                           channel_multiplier=0)
            iota_b = cp.tile([128, BMAX * 128], dt.bfloat16)
            nc.vector.tensor_copy(iota_b[:], iota_i[:])
            # weight tiles load interleaved with the first groups' xg (below):
            # stage 2 of group 0 is the first consumer, and loading them all
            # up front would delay stage 1
            xT_sb = cp.tile([128, 2 * NPAD], dt.bfloat16)
            w_sbs = []
            for r8 in range(R):
                w_r = cp.tile([128, 2 * OUT], dt.bfloat16, name=f"w_r{r8}")
                w_sbs.append(w_r)
            root_sb = cp.tile([128, 2 * OUT], dt.bfloat16)

            def stage2(g, st):
                out_ps = pso.tile([128, OUT], dt.float32, tag="ops")
                out_sb = outp.tile([128, OUT], dt.float32, tag="osb")
                for (fo, fl) in ((0, 512), (512, 288)):
                    for h in range(2):
                        nc.tensor.matmul(
                            out=out_ps[:, fo:fo + fl],
                            lhsT=xT_sb[:, h * NPAD + g * 128: h * NPAD + (g + 1) * 128],
                            rhs=root_sb[:, h * OUT + fo: h * OUT + fo + fl],
                            start=(h == 0), stop=False)
                    for r8 in range(R):
                        for h in range(2):
                            nc.tensor.matmul(
                                out=out_ps[:, fo:fo + fl],
                                lhsT=st[:, h * 1024 + r8 * 128: h * 1024 + (r8 + 1) * 128],
                                rhs=w_sbs[r8][:, h * OUT + fo: h * OUT + fo + fl],
                                start=False, stop=(r8 == R - 1 and h == 1))
                nc.scalar.copy(out_sb[:], out_ps[:])
                nc.sync.dma_start(out_d[g * 128:(g + 1) * 128, :], out_sb[:])

            gbase = 0  # running chunk index
            prev = None  # (g, st) awaiting stage2, one group behind
            for g in range(GROUPS):
                bchunks = chunks[g * 8:(g + 1) * 8]
                gch = gchs[g]
                # gathered source rows, slot-ordered: partition = slot % 128.
                # One tile per DMA split so stage-1 matmuls only wait for the
                # split they read, not the whole group load.
                nsplit = 8
                csz = -(-gch // nsplit)
                xgs = []
                for si in range(nsplit):
                    c0, c1 = si * csz, min((si + 1) * csz, gch)
                    if c0 >= c1:
                        break
                    xs = xgp.tile([128, (c1 - c0) * IN], dt.bfloat16, tag=f"xg{si}")
                    nc.sync.dma_start(
                        xs[:], xg_d[:, (gbase + c0) * IN:(gbase + c1) * IN])
                    xgs.append(xs)
                if g == 0:
                    nc.sync.dma_start(xT_sb[:], xT_d[:, :])
                    nc.sync.dma_start(root_sb[:], root_d[:, :])
                    for r8 in range(R):
                        nc.sync.dma_start(
                            w_sbs[r8][:],
                            w_d[:, r8 * 2 * OUT:(r8 + 1) * 2 * OUT])

                def xg_slice(ci, h):
                    si, cl = ci // csz, ci % csz
                    return xgs[si][:, cl * IN + h * 128: cl * IN + (h + 1) * 128]
                # st columns: h*1024 + r*128 + b8*16 + n  (h = input half)
                st = stp.tile([128, 2048], dt.bfloat16, tag="st")
                st5 = st[:].rearrange("p (h r bb n) -> p h r bb n", h=2, r=8, bb=8, n=16)
                coff = 0
                for b8 in range(8):
                    cb = bchunks[b8]
                    # per-block one-hot: oh[p, c, f] = (segl[p, coff+c] == f)
                    oh = ohp.tile([128, cb * 128], dt.bfloat16, tag=f"oh{b8}")
                    nc.vector.tensor_tensor(
                        out=oh[:].rearrange("p (c f) -> p c f", f=128),
                        in0=segl_sb[:, gbase + coff:gbase + coff + cb]
                            .to_broadcast([128, cb, 128]),
                        in1=iota_b[:, :cb * 128].rearrange("p (c f) -> p c f", f=128),
                        op=mybir.AluOpType.is_equal)
                    pss = []
                    for h in range(2):
                        ps_h = psp.tile([128, 128], dt.float32, tag=f"ps{h}")
                        pss.append(ps_h)
                    for ci in range(coff, coff + cb):
                        for h in range(2):
                            nc.tensor.matmul(
                                out=pss[h][:],
                                lhsT=xg_slice(ci, h),
                                rhs=oh[:, (ci - coff) * 128:(ci - coff + 1) * 128],
                                start=(ci == coff), stop=(ci == coff + cb - 1))
                    for h in range(2):
                        nc.scalar.copy(
                            st5[:, h, :, b8, :],
                            pss[h][:].rearrange("p (r n) -> p r n", r=8, n=16))
                    coff += cb
                # groups 0-1 run stage 2 immediately (pipeline still filling);
                # later groups lag one group so PE never waits on the copies
                if g < 2:
                    stage2(g, st)
                else:
                    if prev is not None:
                        stage2(*prev)
                    prev = (g, st)
                gbase += gch
            if prev is not None:
                stage2(*prev)
    nc.compile()
    return nc


def _prep_core(src, dst, et, core):
    """Per-core block permutation + sorted schedule ingredients."""
    dl = dst - core * NPC
    blk = (dl // 16).astype(np.int64)
    counts = np.bincount(blk, minlength=BLOCKS)
    # descending count: big stage-1 groups first keep PE fed during the
    # DMA pipeline fill; ranks align across cores either way
    perm = np.argsort(-counts, kind="stable")
    rank = np.empty(BLOCKS, np.int64)
    rank[perm] = np.arange(BLOCKS)
    key = rank[blk] * 128 + et.astype(np.int64) * 16 + (dl % 16)
    order = np.argsort(key, kind="stable")
    return counts[perm], perm, key, order


def kernel(x, edge_index, edge_type, bases, att, root, bias):
    global LAST_RESULT
    x = np.asarray(x, dtype=np.float32)
    edge_index = np.asarray(edge_index, dtype=np.int32)
    edge_type = np.asarray(edge_type, dtype=np.int32)
    bases = np.asarray(bases, dtype=np.float32)
    att = np.asarray(att, dtype=np.float32)
    root = np.asarray(root, dtype=np.float32)
    bias = np.asarray(bias, dtype=np.float32)

    src_all, dst_all = edge_index[0], edge_index[1]
    core_of = dst_all // NPC

    per_core, preps = [], []
    for c in range(NCORES):
        m = core_of == c
        pc = (src_all[m], dst_all[m], edge_type[m])
        per_core.append(pc)
        preps.append(_prep_core(pc[0], pc[1], pc[2], c))
    counts_sched = np.stack([p[0] for p in preps])          # [8, BLOCKS] desc
    chunks = np.maximum(1, -(-counts_sched // CHUNK)).max(0)
    chunks = tuple(int(v) for v in chunks)
    TOT = sum(chunks) * CHUNK
    block_slot_start = np.concatenate([[0], np.cumsum(np.asarray(chunks) * CHUNK)])

    key = (tuple(chunks),)
    if key not in _PROGRAM_CACHE:
        _PROGRAM_CACHE[key] = _build(chunks)
    nc = _PROGRAM_CACHE[key]

    # shared weights
    W = np.einsum("rb,bio->rio", att, bases).astype(np.float32)      # [R, IN, OUT]
    w_dev = np.ascontiguousarray(
        W.reshape(R, 2, 128, OUT).transpose(2, 0, 1, 3).reshape(128, R * 2 * OUT)
    ).astype(BF16)
    root_dev = np.ascontiguousarray(
        root.reshape(2, 128, OUT).transpose(1, 0, 2).reshape(128, 2 * OUT)
    ).astype(BF16)
    x_bf = np.ascontiguousarray(x).astype(BF16)

    in_maps, perms = [], []
    for c in range(NCORES):
        src, dst, et = per_core[c]
        counts_s, perm, key_s, order = preps[c]
        perms.append(perm)
        src_s = src[order]
        key_ss = key_s[order]
        slot16 = key_ss // 128                     # schedule position of block
        segl_s = (key_ss % 128).astype(np.float32)
        edge_block_start = np.concatenate([[0], np.cumsum(counts_s)])
        within = np.arange(len(src_s)) - edge_block_start[slot16]
        pos = block_slot_start[slot16] + within

        src_slots = np.zeros(TOT, np.int32)
        segl_slots = np.full(TOT, -1.0, np.float32)
        src_slots[pos] = src_s
        segl_slots[pos] = segl_s

        # partition-major layout: xg_dev[p, c*IN:(c+1)*IN] = x[src of slot c*128+p]
        xg_dev = x_bf[src_slots.reshape(-1, 128).T].reshape(128, -1)
        segl_dev = np.ascontiguousarray(
            segl_slots.reshape(-1, 128).T).astype(BF16)

        # xT rows follow the block schedule order (perm)
        pexp = (perm[:, None] * 16 + np.arange(16)[None, :]).ravel()  # [NPAD]
        xc = np.zeros((NPAD, IN), np.float32)
        xc[:NPC] = x[c * NPC:(c + 1) * NPC]
        xcp = xc[pexp]
        xT_dev = np.ascontiguousarray(
            xcp.reshape(NPAD, 2, 128).transpose(2, 1, 0).reshape(128, 2 * NPAD)
        ).astype(BF16)

        in_maps.append({
            "xg": xg_dev, "segl": segl_dev,
            "xT": xT_dev, "w": w_dev, "root": root_dev,
        })

    res = run_bass_kernel_spmd(nc, in_maps, core_ids=list(range(NCORES)))
    LAST_RESULT = res

    out = np.empty((N, OUT), np.float32)
    for c in range(NCORES):
        pexp = (perms[c][:, None] * 16 + np.arange(16)[None, :]).ravel()
        rows = res.results[c]["out"]               # [NPAD, OUT], schedule order
        mask = pexp < NPC
        out[c * NPC + pexp[mask]] = rows[mask]
    out += bias[None, :]
    return out


# revision 54
# speedup vs baseline: 1.0233x; 1.0153x over previous
"""RGCN (basis-decomposed relational GCN) forward on 8 Trainium2 NeuronCores.

Strategy: shard by destination node (2500 nodes/core). Host buckets+sorts each
core's incoming edges by (dst-block, relation, dst-within-block) and gathers
the source rows into slot order (bf16), so the device streams them densely --
no on-device gather (SWDGE dma_gather costs ~8 ns/row serial on GpSimd, a
~640 us/core floor). Blocks are processed in per-core descending-edge-count
order so all cores share a tight chunk schedule (max of order statistics
instead of elementwise max). On device, per 128-segment block, streamed rows
are scatter-added into PSUM via one-hot matmuls (one-hots built in one
is_equal per group from a broadcast segment-label read); a second fused
matmul stage, software-pipelined one group behind, contracts the
per-(node,relation) sums with the relation weights plus the root term.
No collectives: each core owns its output rows outright.
"""

import os
import sys

import numpy as np
import ml_dtypes

for _p in ("/opt/trn_rl_repo", "/root/.axon_site/_ro/trn_rl_repo"):
    if os.path.isdir(_p) and _p not in sys.path:
        sys.path.append(_p)

import concourse.bacc as bacc
import concourse.tile as tile
from concourse import mybir
from concourse.bass_utils import run_bass_kernel_spmd

BF16 = ml_dtypes.bfloat16
N, E, IN, OUT, R = 20000, 640000, 256, 800, 8
NCORES = 8
NPC = N // NCORES            # 2500 nodes per core
NPAD = 2560                  # padded to 20 groups of 128 nodes
BLOCKS = NPAD // 16          # 160 blocks of 16 nodes (= 128 segments each)
GROUPS = NPAD // 128         # 20
CHUNK = 128

_PROGRAM_CACHE = {}
LAST_RESULT = None           # test harness reads profiling info from here


def _build(chunks):
    """Compile the SPMD program for per-slot chunk counts (same on all cores)."""
    dt = mybir.dt
    nc = bacc.Bacc("TRN2", target_bir_lowering=False, debug=False,
                   enable_asserts=True, num_devices=NCORES)
    TOT = sum(chunks) * CHUNK
    gchs = [sum(chunks[g * 8:(g + 1) * 8]) for g in range(GROUPS)]
    GMAX = max(gchs)
    xg_d = nc.dram_tensor("xg", [128, (TOT // 128) * IN], dt.bfloat16,
                          kind="ExternalInput").ap()
    segl_d = nc.dram_tensor("segl", [128, TOT // 128], dt.bfloat16, kind="ExternalInput").ap()
    xT_d = nc.dram_tensor("xT", [128, 2 * NPAD], dt.bfloat16, kind="ExternalInput").ap()
    w_d = nc.dram_tensor("w", [128, R * 2 * OUT], dt.bfloat16, kind="ExternalInput").ap()
    root_d = nc.dram_tensor("root", [128, 2 * OUT], dt.bfloat16, kind="ExternalInput").ap()
    out_d = nc.dram_tensor("out", [NPAD, OUT], dt.float32, kind="ExternalOutput").ap()

    with tile.TileContext(nc) as tc:
        with tc.tile_pool(name="const", bufs=1) as cp, \
             tc.tile_pool(name="xgp", bufs=3) as xgp, \
             tc.tile_pool(name="ohp", bufs=3) as ohp, \
             tc.tile_pool(name="stp", bufs=3) as stp, \
             tc.tile_pool(name="outp", bufs=3) as outp, \
             tc.tile_pool(name="psp", bufs=2, space="PSUM") as psp, \
             tc.tile_pool(name="pso", bufs=2, space="PSUM") as pso:
            # segl + iota first: the group-0 one-hot depends only on these
            segl_sb = cp.tile([128, TOT // 128], dt.bfloat16)
            nc.sync.dma_start(segl_sb[:], segl_d[:, :])
            # iota_b[p, c*128+f] = f (contiguous, one block's worth), for is_equal
            BMAX = max(chunks)
            iota_i = cp.tile([128, BMAX * 128], dt.int32)
            nc.gpsimd.iota(iota_i[:], pattern=[[0, BMAX], [1, 128]], base=0,
                           channel_multiplier=0)
            iota_b = cp.tile([128, BMAX * 128], dt.bfloat16)
            nc.vector.tensor_copy(iota_b[:], iota_i[:])
            # weight tiles load interleaved with the first groups' xg (below):
            # stage 2 of group 0 is the first consumer, and loading them all
            # up front would delay stage 1
            xT_sb = cp.tile([128, 2 * NPAD], dt.bfloat16)
            w_sbs = []
            for r8 in range(R):
                w_r = cp.tile([128, 2 * OUT], dt.bfloat16, name=f"w_r{r8}")
                w_sbs.append(w_r)
            root_sb = cp.tile([128, 2 * OUT], dt.bfloat16)

            def stage2(g, st):
                out_ps = pso.tile([128, OUT], dt.float32, tag="ops")
                out_sb = outp.tile([128, OUT], dt.float32, tag="osb")
                # lhsT-major order: both fo tiles consume the same stationary
                # weights back-to-back, maximizing PE weight reuse adjacency
                for h in range(2):
                    for (fo, fl) in ((0, 512), (512, 288)):
                        nc.tensor.matmul(
                            out=out_ps[:, fo:fo + fl],
                            lhsT=xT_sb[:, h * NPAD + g * 128: h * NPAD + (g + 1) * 128],
                            rhs=root_sb[:, h * OUT + fo: h * OUT + fo + fl],
                            start=(h == 0), stop=False)
                for r8 in range(R):
                    for h in range(2):
                        for (fo, fl) in ((0, 512), (512, 288)):
                            nc.tensor.matmul(
                                out=out_ps[:, fo:fo + fl],
                                lhsT=st[:, h * 1024 + r8 * 128: h * 1024 + (r8 + 1) * 128],
                                rhs=w_sbs[r8][:, h * OUT + fo: h * OUT + fo + fl],
                                start=False, stop=(r8 == R - 1 and h == 1))
                nc.scalar.copy(out_sb[:], out_ps[:])
                nc.sync.dma_start(out_d[g * 128:(g + 1) * 128, :], out_sb[:])

            gbase = 0  # running chunk index
            prev = None  # (g, st) awaiting stage2, one group behind
            for g in range(GROUPS):
                bchunks = chunks[g * 8:(g + 1) * 8]
                gch = gchs[g]
                # gathered source rows, slot-ordered: partition = slot % 128.
                # One tile per DMA split so stage-1 matmuls only wait for the
                # split they read, not the whole group load.
                nsplit = 8
                csz = -(-gch // nsplit)
                xgs = []
                for si in range(nsplit):
                    c0, c1 = si * csz, min((si + 1) * csz, gch)
                    if c0 >= c1:
                        break
                    xs = xgp.tile([128, (c1 - c0) * IN], dt.bfloat16, tag=f"xg{si}")
                    nc.sync.dma_start(
                        xs[:], xg_d[:, (gbase + c0) * IN:(gbase + c1) * IN])
                    xgs.append(xs)
                if g == 0:
                    nc.sync.dma_start(xT_sb[:], xT_d[:, :])
                    nc.sync.dma_start(root_sb[:], root_d[:, :])
                    for r8 in range(R):
                        nc.sync.dma_start(
                            w_sbs[r8][:],
                            w_d[:, r8 * 2 * OUT:(r8 + 1) * 2 * OUT])

                def xg_slice(ci, h):
                    si, cl = ci // csz, ci % csz
                    return xgs[si][:, cl * IN + h * 128: cl * IN + (h + 1) * 128]
                # st columns: h*1024 + r*128 + b8*16 + n  (h = input half)
                st = stp.tile([128, 2048], dt.bfloat16, tag="st")
                st5 = st[:].rearrange("p (h r bb n) -> p h r bb n", h=2, r=8, bb=8, n=16)
                coff = 0
                for b8 in range(8):
                    cb = bchunks[b8]
                    # per-block one-hot: oh[p, c, f] = (segl[p, coff+c] == f)
                    oh = ohp.tile([128, cb * 128], dt.bfloat16, tag=f"oh{b8}")
                    nc.vector.tensor_tensor(
                        out=oh[:].rearrange("p (c f) -> p c f", f=128),
                        in0=segl_sb[:, gbase + coff:gbase + coff + cb]
                            .to_broadcast([128, cb, 128]),
                        in1=iota_b[:, :cb * 128].rearrange("p (c f) -> p c f", f=128),
                        op=mybir.AluOpType.is_equal)
                    pss = []
                    for h in range(2):
                        ps_h = psp.tile([128, 128], dt.float32, tag=f"ps{h}")
                        pss.append(ps_h)
                    for ci in range(coff, coff + cb):
                        for h in range(2):
                            nc.tensor.matmul(
                                out=pss[h][:],
                                lhsT=xg_slice(ci, h),
                                rhs=oh[:, (ci - coff) * 128:(ci - coff + 1) * 128],
                                start=(ci == coff), stop=(ci == coff + cb - 1))
                    for h in range(2):
                        nc.scalar.copy(
                            st5[:, h, :, b8, :],
                            pss[h][:].rearrange("p (r n) -> p r n", r=8, n=16))
                    coff += cb
                # groups 0-1 run stage 2 immediately (pipeline still filling);
                # later groups lag one group so PE never waits on the copies
                if g < 2:
                    stage2(g, st)
                else:
                    if prev is not None:
                        stage2(*prev)
                    prev = (g, st)
                gbase += gch
            if prev is not None:
                stage2(*prev)
    nc.compile()
    return nc


def _prep_core(src, dst, et, core):
    """Per-core block permutation + sorted schedule ingredients."""
    dl = dst - core * NPC
    blk = (dl // 16).astype(np.int64)
    counts = np.bincount(blk, minlength=BLOCKS)
    # descending count: big stage-1 groups first keep PE fed during the
    # DMA pipeline fill; ranks align across cores either way
    perm = np.argsort(-counts, kind="stable")
    rank = np.empty(BLOCKS, np.int64)
    rank[perm] = np.arange(BLOCKS)
    key = rank[blk] * 128 + et.astype(np.int64) * 16 + (dl % 16)
    order = np.argsort(key, kind="stable")
    return counts[perm], perm, key, order


def kernel(x, edge_index, edge_type, bases, att, root, bias):
    global LAST_RESULT
    x = np.asarray(x, dtype=np.float32)
    edge_index = np.asarray(edge_index, dtype=np.int32)
    edge_type = np.asarray(edge_type, dtype=np.int32)
    bases = np.asarray(bases, dtype=np.float32)
    att = np.asarray(att, dtype=np.float32)
    root = np.asarray(root, dtype=np.float32)
    bias = np.asarray(bias, dtype=np.float32)

    src_all, dst_all = edge_index[0], edge_index[1]
    core_of = dst_all // NPC

    per_core, preps = [], []
    for c in range(NCORES):
        m = core_of == c
        pc = (src_all[m], dst_all[m], edge_type[m])
        per_core.append(pc)
        preps.append(_prep_core(pc[0], pc[1], pc[2], c))
    counts_sched = np.stack([p[0] for p in preps])          # [8, BLOCKS] desc
    chunks = np.maximum(1, -(-counts_sched // CHUNK)).max(0)
    chunks = tuple(int(v) for v in chunks)
    TOT = sum(chunks) * CHUNK
    block_slot_start = np.concatenate([[0], np.cumsum(np.asarray(chunks) * CHUNK)])

    key = (tuple(chunks),)
    if key not in _PROGRAM_CACHE:
        _PROGRAM_CACHE[key] = _build(chunks)
    nc = _PROGRAM_CACHE[key]

    # shared weights
    W = np.einsum("rb,bio->rio", att, bases).astype(np.float32)      # [R, IN, OUT]
    w_dev = np.ascontiguousarray(
        W.reshape(R, 2, 128, OUT).transpose(2, 0, 1, 3).reshape(128, R * 2 * OUT)
    ).astype(BF16)
    root_dev = np.ascontiguousarray(
        root.reshape(2, 128, OUT).transpose(1, 0, 2).reshape(128, 2 * OUT)
    ).astype(BF16)
    x_bf = np.ascontiguousarray(x).astype(BF16)

    in_maps, perms = [], []
    for c in range(NCORES):
        src, dst, et = per_core[c]
        counts_s, perm, key_s, order = preps[c]
        perms.append(perm)
        src_s = src[order]
        key_ss = key_s[order]
        slot16 = key_ss // 128                     # schedule position of block
        segl_s = (key_ss % 128).astype(np.float32)
        edge_block_start = np.concatenate([[0], np.cumsum(counts_s)])
        within = np.arange(len(src_s)) - edge_block_start[slot16]
        pos = block_slot_start[slot16] + within

        src_slots = np.zeros(TOT, np.int32)
        segl_slots = np.full(TOT, -1.0, np.float32)
        src_slots[pos] = src_s
        segl_slots[pos] = segl_s

        # partition-major layout: xg_dev[p, c*IN:(c+1)*IN] = x[src of slot c*128+p]
        xg_dev = x_bf[src_slots.reshape(-1, 128).T].reshape(128, -1)
        segl_dev = np.ascontiguousarray(
            segl_slots.reshape(-1, 128).T).astype(BF16)

        # xT rows follow the block schedule order (perm)
        pexp = (perm[:, None] * 16 + np.arange(16)[None, :]).ravel()  # [NPAD]
        xc = np.zeros((NPAD, IN), np.float32)
        xc[:NPC] = x[c * NPC:(c + 1) * NPC]
        xcp = xc[pexp]
        xT_dev = np.ascontiguousarray(
            xcp.reshape(NPAD, 2, 128).transpose(2, 1, 0).reshape(128, 2 * NPAD)
        ).astype(BF16)

        in_maps.append({
            "xg": xg_dev, "segl": segl_dev,
            "xT": xT_dev, "w": w_dev, "root": root_dev,
        })

    res = run_bass_kernel_spmd(nc, in_maps, core_ids=list(range(NCORES)))
    LAST_RESULT = res

    out = np.empty((N, OUT), np.float32)
    for c in range(NCORES):
        pexp = (perms[c][:, None] * 16 + np.arange(16)[None, :]).ravel()
        rows = res.results[c]["out"]               # [NPAD, OUT], schedule order
        mask = pexp < NPC
        out[c * NPC + pexp[mask]] = rows[mask]
    out += bias[None, :]
    return out
